# revision 2
# baseline (speedup 1.0000x reference)
"""Trainium2 Bass kernel for nn_BinaryDense: y = x @ binarize(w).T   [final: v6]

x: [8192, 4096] f32, weight: [4096, 4096] f32 -> y: [8192, 4096] f32.
binarize(w) = +1 if w > 2**-24 else -1 (matches reference round-half-even).

Strategy (8 cores), v5:
  - data-parallel over x rows; weight prep sharded by contraction dim
    (core c owns w[:, c*512:(c+1)*512]); per-o-block AllGathers.
  - the first o-block's AllGather is split into two 2MB halves so the
    first weight chunk lands as early as possible; matmul group 0
    consumes (blk, itl 0-1) pairs first to match.
  - binarize via a single Sign activation on the scalar engine
    (w - 2^-24 is never exactly 0 for the fp32 grid in practice).
  - phase-C weight stream rides gpsimd SWDGE so the Tile scheduler's
    DMAHW semaphore lanes contain only prep DMAs — avoids a false
    PE-queue wait on AllGather-gated weight DMAs (measured 30-90us
    stall in v2).
  - w and x loads interleave on the sync HWDGE queue; PSUM transpose
    drains alternate between vector and scalar engines.
"""

import numpy as np

import concourse.bass as bass
import concourse.tile as tile
from concourse import bacc, mybir
from concourse.bass_utils import run_bass_kernel_spmd
from concourse.masks import make_identity

N_CORES = 8
B = 1024
D = 4096
ISH = D // N_CORES  # 512
BT = 128
OT = 512
N_BT = B // BT      # 8
N_OT = D // OT      # 8
N_IT = D // 128     # 32
CK = 2048
XQ = 4

F32 = mybir.dt.float32
F16 = mybir.dt.float16

BIN_THRESH = float(2.0 ** -24)

_CACHED = {}


def _build(repeat=1):
    nc = bacc.Bacc("TRN2", target_bir_lowering=False, debug=False,
                   num_devices=N_CORES)
    x = nc.dram_tensor("x", [B, D], F32, kind="ExternalInput").ap()
    wsh = nc.dram_tensor("wsh", [D, ISH], F32, kind="ExternalInput").ap()
    y = nc.dram_tensor("y", [B, D], F32, kind="ExternalOutput").ap()
    # per-o-block transposed binarized shard; block 0 split in two i-halves
    wshT_a = nc.dram_tensor("wshT_a", [ISH // 2, OT], F16).ap()
    wshT_b = nc.dram_tensor("wshT_b", [ISH // 2, OT], F16).ap()
    wshT_o = [None] + [
        nc.dram_tensor(f"wshT_o{ot}", [ISH, OT], F16).ap()
        for ot in range(1, N_OT)
    ]
    wT_a = nc.dram_tensor("wT_a", [N_CORES, ISH // 2, OT], F16,
                          addr_space="Shared").ap()
    wT_b = nc.dram_tensor("wT_b", [N_CORES, ISH // 2, OT], F16,
                          addr_space="Shared").ap()
    wT_o = [None] + [
        nc.dram_tensor(f"wT_o{ot}", [N_CORES, ISH, OT], F16,
                       addr_space="Shared").ap()
        for ot in range(1, N_OT)
    ]

    SIGN = mybir.ActivationFunctionType.Sign
    COPY = mybir.ActivationFunctionType.Copy
    RG = [list(range(N_CORES))]

    with tile.TileContext(nc) as tc:
      for _rep in range(repeat):
        with (
            tc.tile_pool(name="const", bufs=1) as const,
            tc.tile_pool(name="prep", bufs=3) as prep,
            tc.tile_pool(name="xres", bufs=1) as xres,
            tc.tile_pool(name="wmov", bufs=16) as wmov,
            tc.tile_pool(name="drain", bufs=6) as drain,
        ):
            id16 = const.tile([128, 128], F16, tag="id16")
            make_identity(nc, id16[:])
            id32 = const.tile([128, 128], F32, tag="id32")
            make_identity(nc, id32[:])
            bsn = const.tile([128, 1], F32, tag="bsn")
            nc.gpsimd.memset(bsn[:], -BIN_THRESH)

            xthi = [
                xres.tile([128, (N_IT // XQ) * B], F16, tag=f"xthi{q}",
                          name=f"xthi{q}")
                for q in range(XQ)
            ]

            def ag(ins_ap, outs_ap):
                nc.gpsimd.collective_compute(
                    "AllGather", mybir.AluOpType.bypass,
                    replica_groups=RG, ins=[ins_ap], outs=[outs_ap])

            with tc.tile_pool(name="tpsum", bufs=3, space="PSUM") as tpsum:
                # ---- w loads + first x loads interleaved on sync;
                # only 4 x prefetches (xa bufs=6: deeper would deadlock
                # the sync FIFO on buffer-release WARs) ----
                was = {}
                xas = {}
                for ot in range(N_OT):
                    for rt in range(4):
                        wa = prep.tile([128, ISH], F32, tag="wa", bufs=8)
                        nc.sync.dma_start(
                            wa[:], wsh[bass.ts(ot * 4 + rt, 128), :])
                        was[(ot, rt)] = wa
                    if ot % 2 == 0:
                        ck, bt = divmod(ot // 2, N_BT)
                        xa = prep.tile([128, CK], F32, tag="xa", bufs=6)
                        nc.sync.dma_start(
                            xa[:], x[bass.ts(bt, BT), bass.ts(ck, CK)])
                        xas[(ck, bt)] = xa

                # ---- binarize upfront (scalar Sign, paced by loads) ----
                wbs = {}
                for ot in range(N_OT):
                    for rt in range(4):
                        wb = prep.tile([128, ISH], F16, tag="wb", bufs=8)
                        nc.scalar.activation(
                            wb[:], was[(ot, rt)][:], SIGN, bias=bsn[:])
                        wbs[(ot, rt)] = wb

                # ---- w-prep per o-block: transpose (PE), drain
                # (vector), store (scalar) + AG ----
                for ot in range(N_OT):
                    for itl in range(4):
                        twp = tpsum.tile([128, OT], F16, tag="twp")
                        for rt in range(4):
                            nc.tensor.transpose(
                                twp[:, bass.ts(rt, 128)],
                                wbs[(ot, rt)][:, bass.ts(itl, 128)], id16[:])
                        wsb = prep.tile([128, OT], F16, tag="wsb", bufs=4)
                        nc.vector.tensor_copy(wsb[:], twp[:])
                        if ot == 0:
                            half, itl2 = divmod(itl, 2)
                            tgt = wshT_b if half else wshT_a
                            nc.scalar.dma_start(
                                tgt[bass.ts(itl2, 128), :], wsb[:])
                            if itl == 1:
                                ag(wshT_a[:], wT_a[:])
                            elif itl == 3:
                                ag(wshT_b[:], wT_b[:])
                        else:
                            nc.scalar.dma_start(
                                wshT_o[ot][bass.ts(itl, 128), :], wsb[:])
                    if ot > 0:
                        ag(wshT_o[ot][:], wT_o[ot][:])

                # ---- x-prep: transpose to resident fp16 ----
                nd = 0
                for ck in range(D // CK):
                    for bt in range(N_BT):
                        if (ck, bt) in xas:
                            xa = xas[(ck, bt)]
                        else:
                            xa = prep.tile([128, CK], F32, tag="xa", bufs=6)
                            nc.sync.dma_start(
                                xa[:], x[bass.ts(bt, BT), bass.ts(ck, CK)])
                        for k in range(CK // OT):
                            txp = tpsum.tile([128, OT], F32, tag="txp",
                                             bufs=5)
                            for j in range(4):
                                itl = k * 4 + j
                                nc.tensor.transpose(
                                    txp[:, bass.ts(j, 128)],
                                    xa[:, bass.ts(itl, 128)], id32[:])
                            it0 = ck * (CK // 128) + k * 4
                            q, itq0 = divmod(it0, N_IT // XQ)
                            dst = xthi[q][:, bass.ds(bt * B + itq0 * 128, OT)]
                            if nd % 2 == 0:
                                nc.scalar.activation(dst, txp[:], COPY)
                            else:
                                nc.vector.tensor_copy(dst, txp[:])
                            nd += 1

            # ---- phase C ----
            with tc.tile_pool(name="psum", bufs=8, space="PSUM") as psum:
                for ot in range(N_OT):
                    pts = []
                    for bt in range(N_BT):
                        pt = psum.tile([128, OT], F32, tag="acc")
                        pts.append(pt)
                    if ot == 0:
                        order = ([(blk, itl) for itl2 in range(2)
                                  for blk in range(N_CORES)
                                  for itl in (itl2 * 2, itl2 * 2 + 1)])
                    else:
                        order = [(it // 4, it % 4) for it in range(N_IT)]
                    for pos, (blk, itl) in enumerate(order):
                        it = blk * 4 + itl
                        wt = wmov.tile([128, OT], F16, tag="wmov")
                        if ot == 0:
                            half, itl2 = divmod(itl, 2)
                            src = (wT_b if half else wT_a)[
                                blk, bass.ts(itl2, 128), :]
                        else:
                            src = wT_o[ot][blk, bass.ts(itl, 128), :]
                        nc.gpsimd.dma_start(wt[:], src)
                        q, itq = divmod(it, N_IT // XQ)
                        for bt in range(N_BT):
                            nc.tensor.matmul(
                                pts[bt][:],
                                xthi[q][:, bass.ds(bt * B + itq * BT, BT)],
                                wt[:],
                                start=(pos == 0), stop=(pos == N_IT - 1))
                    for bt in range(N_BT):
                        st = drain.tile([128, OT], F32, tag="drain")
                        nc.vector.tensor_copy(st[:], pts[bt][:])
                        nc.scalar.dma_start(
                            y[bass.ts(bt, BT), bass.ts(ot, OT)], st[:])

    nc.finalize()
    return nc


def _get_nc():
    if "nc" not in _CACHED:
        _CACHED["nc"] = _build()
    return _CACHED["nc"]


def build_nc(repeat=1, **kw):
    return _build(repeat=repeat, **kw)


def run(x, weight, **run_kwargs):
    nc = _get_nc()
    x = np.ascontiguousarray(x, dtype=np.float32)
    weight = np.ascontiguousarray(weight, dtype=np.float32)
    in_maps = [
        {"x": x[c * B:(c + 1) * B],
         "wsh": np.ascontiguousarray(weight[:, c * ISH:(c + 1) * ISH])}
        for c in range(N_CORES)
    ]
    res = run_bass_kernel_spmd(nc, in_maps, list(range(N_CORES)), **run_kwargs)
    out = np.concatenate([res.results[c]["y"] for c in range(N_CORES)], axis=0)
    return out, res


def kernel(x, weight):
    out, _ = run(x, weight)
    return out


# revision 3
# speedup vs baseline: 1.0952x; 1.0952x over previous
"""Trainium2 Bass kernel for nn_BinaryDense: y = x @ binarize(w).T   [final: v9, fp8 weight gather]

x: [8192, 4096] f32, weight: [4096, 4096] f32 -> y: [8192, 4096] f32.
binarize(w) = +1 if w > 2**-24 else -1 (matches reference round-half-even).

Strategy (8 cores), v5:
  - data-parallel over x rows; weight prep sharded by contraction dim
    (core c owns w[:, c*512:(c+1)*512]); per-o-block AllGathers.
  - the first o-block's AllGather is split into two 2MB halves so the
    first weight chunk lands as early as possible; matmul group 0
    consumes (blk, itl 0-1) pairs first to match.
  - binarize via a single Sign activation on the scalar engine
    (w - 2^-24 is never exactly 0 for the fp32 grid in practice).
  - phase-C weight stream rides gpsimd SWDGE so the Tile scheduler's
    DMAHW semaphore lanes contain only prep DMAs — avoids a false
    PE-queue wait on AllGather-gated weight DMAs (measured 30-90us
    stall in v2).
  - w and x loads interleave on the sync HWDGE queue; PSUM transpose
    drains alternate between vector and scalar engines.
"""

import numpy as np

import concourse.bass as bass
import concourse.tile as tile
from concourse import bacc, mybir
from concourse.bass_utils import run_bass_kernel_spmd
from concourse.masks import make_identity

N_CORES = 8
B = 1024
D = 4096
ISH = D // N_CORES  # 512
BT = 128
OT = 512
N_BT = B // BT      # 8
N_OT = D // OT      # 8
N_IT = D // 128     # 32
CK = 2048
XQ = 4

F32 = mybir.dt.float32
F16 = mybir.dt.float16
F8 = mybir.dt.float8e4

BIN_THRESH = float(2.0 ** -24)

_CACHED = {}


def _build(repeat=1):
    nc = bacc.Bacc("TRN2", target_bir_lowering=False, debug=False,
                   num_devices=N_CORES)
    x = nc.dram_tensor("x", [B, D], F32, kind="ExternalInput").ap()
    wsh = nc.dram_tensor("wsh", [D, ISH], F32, kind="ExternalInput").ap()
    y = nc.dram_tensor("y", [B, D], F32, kind="ExternalOutput").ap()
    # per-o-block transposed binarized shard; block 0 split in two i-halves
    wshT_a = nc.dram_tensor("wshT_a", [ISH // 2, OT], F8).ap()
    wshT_b = nc.dram_tensor("wshT_b", [ISH // 2, OT], F8).ap()
    wshT_o = [None] + [
        nc.dram_tensor(f"wshT_o{ot}", [ISH, OT], F8).ap()
        for ot in range(1, N_OT)
    ]
    wT_a = nc.dram_tensor("wT_a", [N_CORES, ISH // 2, OT], F8,
                          addr_space="Shared").ap()
    wT_b = nc.dram_tensor("wT_b", [N_CORES, ISH // 2, OT], F8,
                          addr_space="Shared").ap()
    wT_o = [None] + [
        nc.dram_tensor(f"wT_o{ot}", [N_CORES, ISH, OT], F8,
                       addr_space="Shared").ap()
        for ot in range(1, N_OT)
    ]

    SIGN = mybir.ActivationFunctionType.Sign
    COPY = mybir.ActivationFunctionType.Copy
    RG = [list(range(N_CORES))]

    with tile.TileContext(nc) as tc:
      for _rep in range(repeat):
        with (
            tc.tile_pool(name="const", bufs=1) as const,
            tc.tile_pool(name="prep", bufs=3) as prep,
            tc.tile_pool(name="xres", bufs=1) as xres,
            tc.tile_pool(name="wmov", bufs=16) as wmov,
            tc.tile_pool(name="drain", bufs=6) as drain,
        ):
            id16 = const.tile([128, 128], F16, tag="id16")
            make_identity(nc, id16[:])
            id32 = const.tile([128, 128], F32, tag="id32")
            make_identity(nc, id32[:])
            bsn = const.tile([128, 1], F32, tag="bsn")
            nc.gpsimd.memset(bsn[:], -BIN_THRESH)

            xthi = [
                xres.tile([128, (N_IT // XQ) * B], F16, tag=f"xthi{q}",
                          name=f"xthi{q}")
                for q in range(XQ)
            ]

            def ag(ins_ap, outs_ap):
                nc.gpsimd.collective_compute(
                    "AllGather", mybir.AluOpType.bypass,
                    replica_groups=RG, ins=[ins_ap], outs=[outs_ap])

            with tc.tile_pool(name="tpsum", bufs=3, space="PSUM") as tpsum:
                # ---- w loads + first x loads interleaved on sync;
                # only 4 x prefetches (xa bufs=6: deeper would deadlock
                # the sync FIFO on buffer-release WARs) ----
                was = {}
                xas = {}
                for ot in range(N_OT):
                    for rt in range(4):
                        wa = prep.tile([128, ISH], F32, tag="wa", bufs=8)
                        nc.sync.dma_start(
                            wa[:], wsh[bass.ts(ot * 4 + rt, 128), :])
                        was[(ot, rt)] = wa
                    if ot % 2 == 0:
                        ck, bt = divmod(ot // 2, N_BT)
                        xa = prep.tile([128, CK], F32, tag="xa", bufs=6)
                        nc.sync.dma_start(
                            xa[:], x[bass.ts(bt, BT), bass.ts(ck, CK)])
                        xas[(ck, bt)] = xa

                # ---- binarize upfront (scalar Sign, paced by loads) ----
                wbs = {}
                for ot in range(N_OT):
                    for rt in range(4):
                        wb = prep.tile([128, ISH], F16, tag="wb", bufs=8)
                        nc.scalar.activation(
                            wb[:], was[(ot, rt)][:], SIGN, bias=bsn[:])
                        wbs[(ot, rt)] = wb

                # ---- w-prep per o-block: transpose (PE), drain
                # (vector), store (scalar) + AG ----
                for ot in range(N_OT):
                    for itl in range(4):
                        twp = tpsum.tile([128, OT], F16, tag="twp")
                        for rt in range(4):
                            nc.tensor.transpose(
                                twp[:, bass.ts(rt, 128)],
                                wbs[(ot, rt)][:, bass.ts(itl, 128)], id16[:])
                        wsb = prep.tile([128, OT], F8, tag="wsb", bufs=4)
                        nc.vector.tensor_copy(wsb[:], twp[:])
                        if ot == 0:
                            half, itl2 = divmod(itl, 2)
                            tgt = wshT_b if half else wshT_a
                            nc.scalar.dma_start(
                                tgt[bass.ts(itl2, 128), :], wsb[:])
                            if itl == 1:
                                ag(wshT_a[:], wT_a[:])
                            elif itl == 3:
                                ag(wshT_b[:], wT_b[:])
                        else:
                            nc.scalar.dma_start(
                                wshT_o[ot][bass.ts(itl, 128), :], wsb[:])
                    if ot > 0:
                        ag(wshT_o[ot][:], wT_o[ot][:])

                # ---- x-prep: transpose to resident fp16 ----
                nd = 0
                for ck in range(D // CK):
                    for bt in range(N_BT):
                        if (ck, bt) in xas:
                            xa = xas[(ck, bt)]
                        else:
                            xa = prep.tile([128, CK], F32, tag="xa", bufs=6)
                            nc.sync.dma_start(
                                xa[:], x[bass.ts(bt, BT), bass.ts(ck, CK)])
                        for k in range(CK // OT):
                            txp = tpsum.tile([128, OT], F32, tag="txp",
                                             bufs=5)
                            for j in range(4):
                                itl = k * 4 + j
                                nc.tensor.transpose(
                                    txp[:, bass.ts(j, 128)],
                                    xa[:, bass.ts(itl, 128)], id32[:])
                            it0 = ck * (CK // 128) + k * 4
                            q, itq0 = divmod(it0, N_IT // XQ)
                            dst = xthi[q][:, bass.ds(bt * B + itq0 * 128, OT)]
                            if nd % 2 == 0:
                                nc.scalar.activation(dst, txp[:], COPY)
                            else:
                                nc.vector.tensor_copy(dst, txp[:])
                            nd += 1

            # ---- phase C ----
            with tc.tile_pool(name="psum", bufs=8, space="PSUM") as psum:
                for ot in range(N_OT):
                    pts = []
                    for bt in range(N_BT):
                        pt = psum.tile([128, OT], F32, tag="acc")
                        pts.append(pt)
                    if ot == 0:
                        order = ([(blk, itl) for itl2 in range(2)
                                  for blk in range(N_CORES)
                                  for itl in (itl2 * 2, itl2 * 2 + 1)])
                    else:
                        order = [(it // 4, it % 4) for it in range(N_IT)]
                    for pos, (blk, itl) in enumerate(order):
                        it = blk * 4 + itl
                        wt = wmov.tile([128, OT], F16, tag="wmov")
                        if ot == 0:
                            half, itl2 = divmod(itl, 2)
                            src = (wT_b if half else wT_a)[
                                blk, bass.ts(itl2, 128), :]
                        else:
                            src = wT_o[ot][blk, bass.ts(itl, 128), :]
                        nc.gpsimd.dma_start(wt[:], src)
                        q, itq = divmod(it, N_IT // XQ)
                        for bt in range(N_BT):
                            nc.tensor.matmul(
                                pts[bt][:],
                                xthi[q][:, bass.ds(bt * B + itq * BT, BT)],
                                wt[:],
                                start=(pos == 0), stop=(pos == N_IT - 1))
                    for bt in range(N_BT):
                        st = drain.tile([128, OT], F32, tag="drain")
                        nc.scalar.activation(st[:], pts[bt][:], COPY)
                        nc.scalar.dma_start(
                            y[bass.ts(bt, BT), bass.ts(ot, OT)], st[:])

    nc.finalize()
    return nc


def _get_nc():
    if "nc" not in _CACHED:
        _CACHED["nc"] = _build()
    return _CACHED["nc"]


def build_nc(repeat=1, **kw):
    return _build(repeat=repeat, **kw)


def run(x, weight, **run_kwargs):
    nc = _get_nc()
    x = np.ascontiguousarray(x, dtype=np.float32)
    weight = np.ascontiguousarray(weight, dtype=np.float32)
    in_maps = [
        {"x": x[c * B:(c + 1) * B],
         "wsh": np.ascontiguousarray(weight[:, c * ISH:(c + 1) * ISH])}
        for c in range(N_CORES)
    ]
    res = run_bass_kernel_spmd(nc, in_maps, list(range(N_CORES)), **run_kwargs)
    out = np.concatenate([res.results[c]["y"] for c in range(N_CORES)], axis=0)
    return out, res


def kernel(x, weight):
    out, _ = run(x, weight)
    return out
